# revision 3
# baseline (speedup 1.0000x reference)
"""MoE BERT self-output kernel for 8 Trainium2 NeuronCores.

Math (per batch row b):
    out[b] = LayerNorm(hidden_states[b] @ W[expert_idx[b]] + b[expert_idx[b]]
                       + input_tensor[b]) * gamma + beta

Sharding: data-parallel over the batch dim (16 rows -> 2 rows/core).
On the host we gather each row's expert weight W[expert_idx[b]] and fold the
expert bias into the residual (resid = input_tensor + b[expert_idx]).  Each
core then runs: matmul (f32r on the PE, contraction over H in 8 chunks of
128) + residual add + LayerNorm, fully on-device.

Shapes are hardcoded for E=8, B=16, S=512, H=1024 (fp32).
"""

import numpy as np

import concourse.bacc as bacc
import concourse.tile as tile
from concourse import mybir
from concourse.bass_utils import run_bass_kernel_spmd

E, B, S, H = 8, 16, 512, 1024
N_CORES = 8
R = B // N_CORES  # rows per core = 2
LN_EPS = 1e-12
P = 128
KC = H // P  # 8 contraction chunks
SC = S // P  # 4 output-row chunks
NB = 512     # psum bank free size (fp32)
HB = H // NB  # 2 psum banks per output tile

_CACHE = {}

# module-level knobs used by test.py (harness just calls kernel())
TRACE = False
LAST_RESULT = None


def _build():
    f32 = mybir.dt.float32
    f32r = mybir.dt.float32r

    nc = bacc.Bacc(
        trn_type="TRN2",
        target_bir_lowering=False,
        debug=False,
        num_devices=N_CORES,
    )

    hst_d = nc.dram_tensor("hst", [R, H, S], f32r, kind="ExternalInput").ap()
    w_d = nc.dram_tensor("w", [R, H, H], f32r, kind="ExternalInput").ap()
    resid_d = nc.dram_tensor("resid", [R, S, H], f32, kind="ExternalInput").ap()
    out_d = nc.dram_tensor("out", [R, S, H], f32, kind="ExternalOutput").ap()

    with tile.TileContext(nc) as tc:
        with (
            tc.tile_pool(name="wp", bufs=2) as wp,
            tc.tile_pool(name="hp", bufs=2) as hp,
            tc.tile_pool(name="rp", bufs=3) as rp,
            tc.tile_pool(name="yp", bufs=3) as yp,
            tc.tile_pool(name="st", bufs=8) as st,
            tc.tile_pool(name="singles", bufs=1) as singles,
            tc.tile_pool(name="ps", bufs=2, space="PSUM") as psp,
        ):
            eps_sb = singles.tile([P, 1], f32)
            nc.vector.memset(eps_sb[:], LN_EPS)
            for r in range(R):
                w_sb = wp.tile([P, KC, H], f32r)
                nc.sync.dma_start(
                    out=w_sb[:], in_=w_d[r].rearrange("(kc p) h -> p kc h", p=P)
                )
                hst_sb = hp.tile([P, KC, S], f32r)
                nc.sync.dma_start(
                    out=hst_sb[:], in_=hst_d[r].rearrange("(kc p) s -> p kc s", p=P)
                )
                for sc in range(SC):
                    resid_sb = rp.tile([P, HB, NB], f32)
                    nc.sync.dma_start(
                        out=resid_sb[:],
                        in_=resid_d[r, sc * P : (sc + 1) * P, :].rearrange(
                            "p (hb x) -> p hb x", hb=HB
                        ),
                    )
                    ps = psp.tile([P, HB, NB], f32)
                    for hb in range(HB):
                        for kc in range(KC):
                            nc.tensor.matmul(
                                ps[:, hb, :],
                                lhsT=hst_sb[:, kc, sc * P : (sc + 1) * P],
                                rhs=w_sb[:, kc, hb * NB : (hb + 1) * NB],
                                start=(kc == 0),
                                stop=(kc == KC - 1),
                            )
                    # x = matmul_out + resid (DVE reads PSUM)
                    x_sb = yp.tile([P, HB, NB], f32, tag="x")
                    nc.vector.scalar_tensor_tensor(
                        out=x_sb[:],
                        in0=ps[:],
                        scalar=1.0,
                        in1=resid_sb[:],
                        op0=mybir.AluOpType.mult,
                        op1=mybir.AluOpType.add,
                    )
                    # mean/var over H (two 512-wide bn_stats + aggregate)
                    stats = st.tile([P, HB, 6], f32, tag="stats")
                    for hb in range(HB):
                        nc.vector.bn_stats(out=stats[:, hb, :], in_=x_sb[:, hb, :])
                    mv = st.tile([P, 2], f32, tag="mv")
                    nc.vector.bn_aggr(out=mv[:], in_=stats[:])
                    std = st.tile([P, 1], f32, tag="std")
                    nc.scalar.activation(
                        out=std[:],
                        in_=mv[:, 1:2],
                        func=mybir.ActivationFunctionType.Sqrt,
                        bias=eps_sb[:],
                    )
                    rstd = st.tile([P, 1], f32, tag="rstd")
                    nc.vector.reciprocal(out=rstd[:], in_=std[:])
                    # y = (x - mean) * rstd
                    y_sb = yp.tile([P, HB, NB], f32, tag="y")
                    nc.vector.tensor_scalar(
                        out=y_sb[:],
                        in0=x_sb[:],
                        scalar1=mv[:, 0:1],
                        scalar2=rstd[:],
                        op0=mybir.AluOpType.subtract,
                        op1=mybir.AluOpType.mult,
                    )
                    nc.scalar.dma_start(
                        out=out_d[r, sc * P : (sc + 1) * P, :].rearrange(
                            "p (hb x) -> p hb x", hb=HB
                        ),
                        in_=y_sb[:],
                    )

    nc.compile()
    return nc


def _get_nc():
    if "nc" not in _CACHE:
        _CACHE["nc"] = _build()
    return _CACHE["nc"]


def kernel(hidden_states, input_tensor, expert_idx, W, b, gamma, beta):
    global LAST_RESULT
    hs = np.ascontiguousarray(np.asarray(hidden_states, dtype=np.float32))
    inp = np.ascontiguousarray(np.asarray(input_tensor, dtype=np.float32))
    idx = np.asarray(expert_idx).astype(np.int64)
    W_ = np.asarray(W, dtype=np.float32)
    b_ = np.asarray(b, dtype=np.float32)
    g = np.asarray(gamma, dtype=np.float32)
    be = np.asarray(beta, dtype=np.float32)

    # host-side shard prep: expert gather, bias fold, transpose for the PE
    W_sel = np.ascontiguousarray(W_[idx])                    # [B, H, H]
    resid = inp + b_[idx][:, None, :]                        # [B, S, H]
    hsT = np.ascontiguousarray(hs.transpose(0, 2, 1))        # [B, H, S]

    nc = _get_nc()
    in_maps = [
        {
            "hst": hsT[R * i : R * (i + 1)],
            "w": W_sel[R * i : R * (i + 1)],
            "resid": resid[R * i : R * (i + 1)],
        }
        for i in range(N_CORES)
    ]
    res = run_bass_kernel_spmd(nc, in_maps, list(range(N_CORES)), trace=TRACE)
    LAST_RESULT = res
    out = np.concatenate([res.results[i]["out"] for i in range(N_CORES)], axis=0)

    if not (np.all(g == 1.0) and np.all(be == 0.0)):
        out = out * g + be
    return np.ascontiguousarray(out.astype(np.float32))


# revision 4
# speedup vs baseline: 1.0002x; 1.0002x over previous
"""MoE BERT self-output kernel for 8 Trainium2 NeuronCores.

Math (per batch row b):
    out[b] = LayerNorm(hidden_states[b] @ W[expert_idx[b]] + b[expert_idx[b]]
                       + input_tensor[b]) * gamma + beta

Sharding: data-parallel over the batch dim (16 rows -> 2 rows/core).
On the host we gather each row's expert weight W[expert_idx[b]] and fold the
expert bias into the residual (resid = input_tensor + b[expert_idx]).  Each
core then runs, per row: a [512,1024]x[1024,1024] matmul (f32r on the PE,
contraction over H in 8 chunks of 128), the residual add folded into the
PSUM accumulation via an identity matmul, and LayerNorm read straight out
of PSUM.

Shapes are hardcoded for E=8, B=16, S=512, H=1024 (fp32).
"""

import numpy as np

import concourse.bacc as bacc
import concourse.tile as tile
from concourse import mybir
from concourse.bass_utils import run_bass_kernel_spmd

E, B, S, H = 8, 16, 512, 1024
N_CORES = 8
R = B // N_CORES  # rows per core = 2
LN_EPS = 1e-12
P = 128
KC = H // P  # 8 contraction chunks
SC = S // P  # 4 output-row chunks
NB = 512     # psum bank free size (fp32)
HB = H // NB  # 2 psum banks per output tile

_CACHE = {}

# module-level knobs used by test.py (harness just calls kernel())
TRACE = False
LAST_RESULT = None


def _build():
    f32 = mybir.dt.float32
    f32r = mybir.dt.float32r

    nc = bacc.Bacc(
        trn_type="TRN2",
        target_bir_lowering=False,
        debug=False,
        num_devices=N_CORES,
    )

    hst_d = nc.dram_tensor("hst", [R, KC, P, S], f32r, kind="ExternalInput").ap()
    w_d = nc.dram_tensor("w", [R, KC, P, H], f32r, kind="ExternalInput").ap()
    resid_d = nc.dram_tensor("resid", [R, S, H], f32r, kind="ExternalInput").ap()
    ident_d = nc.dram_tensor("ident", [P, P], f32r, kind="ExternalInput").ap()
    out_d = nc.dram_tensor("out", [R, S, H], f32, kind="ExternalOutput").ap()

    with tile.TileContext(nc) as tc:
        with (
            tc.tile_pool(name="wp", bufs=12) as wp,
            tc.tile_pool(name="hp", bufs=12) as hp,
            tc.tile_pool(name="rp", bufs=4) as rp,
            tc.tile_pool(name="yp", bufs=3) as yp,
            tc.tile_pool(name="st", bufs=8) as st,
            tc.tile_pool(name="singles", bufs=1) as singles,
            tc.tile_pool(name="ps", bufs=3, space="PSUM") as psp,
        ):
            eps_sb = singles.tile([P, 1], f32)
            nc.vector.memset(eps_sb[:], LN_EPS)
            ident_sb = singles.tile([P, P], f32r)
            nc.sync.dma_start(out=ident_sb[:], in_=ident_d[:])

            for r in range(R):
                # per-k-chunk tiles so matmuls start as soon as chunk 0 lands
                w_sb = []
                hst_sb = []
                for kc in range(KC):
                    wt = wp.tile([P, H], f32r, tag="w")
                    nc.sync.dma_start(out=wt[:], in_=w_d[r, kc])
                    w_sb.append(wt)
                    ht = hp.tile([P, S], f32r, tag="h")
                    nc.sync.dma_start(out=ht[:], in_=hst_d[r, kc])
                    hst_sb.append(ht)
                for sc in range(SC):
                    resid_sb = rp.tile([P, HB, NB], f32r)
                    nc.gpsimd.dma_start(
                        out=resid_sb[:],
                        in_=resid_d[r, sc * P : (sc + 1) * P, :].rearrange(
                            "p (hb x) -> p hb x", hb=HB
                        ),
                    )
                    ps = psp.tile([P, HB, NB], f32)
                    for hb in range(HB):
                        for kc in range(KC):
                            nc.tensor.matmul(
                                ps[:, hb, :],
                                lhsT=hst_sb[kc][:, sc * P : (sc + 1) * P],
                                rhs=w_sb[kc][:, hb * NB : (hb + 1) * NB],
                                start=(kc == 0),
                                stop=False,
                            )
                        # += resid (identity matmul closes the accum group)
                        nc.tensor.matmul(
                            ps[:, hb, :],
                            lhsT=ident_sb[:],
                            rhs=resid_sb[:, hb, :],
                            start=False,
                            stop=True,
                        )
                    # mean/var over H straight from PSUM
                    stats = st.tile([P, HB, 6], f32, tag="stats")
                    for hb in range(HB):
                        nc.vector.bn_stats(out=stats[:, hb, :], in_=ps[:, hb, :])
                    mv = st.tile([P, 2], f32, tag="mv")
                    nc.vector.bn_aggr(out=mv[:], in_=stats[:])
                    std = st.tile([P, 1], f32, tag="std")
                    nc.scalar.activation(
                        out=std[:],
                        in_=mv[:, 1:2],
                        func=mybir.ActivationFunctionType.Sqrt,
                        bias=eps_sb[:],
                    )
                    rstd = st.tile([P, 1], f32, tag="rstd")
                    nc.vector.reciprocal(out=rstd[:], in_=std[:])
                    # y = (x - mean) * rstd
                    y_sb = yp.tile([P, HB, NB], f32, tag="y")
                    nc.vector.tensor_scalar(
                        out=y_sb[:],
                        in0=ps[:],
                        scalar1=mv[:, 0:1],
                        scalar2=rstd[:],
                        op0=mybir.AluOpType.subtract,
                        op1=mybir.AluOpType.mult,
                    )
                    nc.scalar.dma_start(
                        out=out_d[r, sc * P : (sc + 1) * P, :].rearrange(
                            "p (hb x) -> p hb x", hb=HB
                        ),
                        in_=y_sb[:],
                    )

    nc.compile()
    return nc


def _get_nc():
    if "nc" not in _CACHE:
        _CACHE["nc"] = _build()
    return _CACHE["nc"]


def kernel(hidden_states, input_tensor, expert_idx, W, b, gamma, beta):
    global LAST_RESULT
    hs = np.ascontiguousarray(np.asarray(hidden_states, dtype=np.float32))
    inp = np.ascontiguousarray(np.asarray(input_tensor, dtype=np.float32))
    idx = np.asarray(expert_idx).astype(np.int64)
    W_ = np.asarray(W, dtype=np.float32)
    b_ = np.asarray(b, dtype=np.float32)
    g = np.asarray(gamma, dtype=np.float32)
    be = np.asarray(beta, dtype=np.float32)

    # host-side shard prep: expert gather, bias fold, transpose for the PE
    # w layout [B, KC, P, H]: w[b, kc, p, h] = W[idx[b], kc*P + p, h]
    W_sel = np.ascontiguousarray(W_[idx]).reshape(B, KC, P, H)
    resid = inp + b_[idx][:, None, :]                        # [B, S, H]
    # hst layout [B, KC, P, S]: hst[b, kc, p, s] = hs[b, s, kc*P + p]
    hsT = np.ascontiguousarray(hs.transpose(0, 2, 1)).reshape(B, KC, P, S)
    ident = np.eye(P, dtype=np.float32)

    nc = _get_nc()
    in_maps = [
        {
            "hst": hsT[R * i : R * (i + 1)],
            "w": W_sel[R * i : R * (i + 1)],
            "resid": resid[R * i : R * (i + 1)],
            "ident": ident,
        }
        for i in range(N_CORES)
    ]
    res = run_bass_kernel_spmd(nc, in_maps, list(range(N_CORES)), trace=TRACE)
    LAST_RESULT = res
    out = np.concatenate([res.results[i]["out"] for i in range(N_CORES)], axis=0)

    if not (np.all(g == 1.0) and np.all(be == 0.0)):
        out = out * g + be
    return np.ascontiguousarray(out.astype(np.float32))
